# revision 5
# baseline (speedup 1.0000x reference)
"""Single-head attention (B=8, S=2048, D=1024) on 8 TRN2 NeuronCores.

Data-parallel over batch: core b handles batch element b entirely.

Key optimizations over the naive dataflow:
  1. Mask sparsity: keys with mask==0 contribute exactly nothing to the
     output (exp(-1e9) == 0 in fp32).  The host gathers the unmasked keys
     (~1024 of 2048) and pads to SK (multiple of 128).  K/V projections,
     scores and PV all shrink ~2x.  Mathematically exact.
  2. Transposed scores: S^T[key, q] = (K^T)^T @ Q^T is computed directly
     with key on the PSUM partition dim.  exp(S^T) then directly yields
     P^T tiles for the PV matmul -- no PE transposes at all -- and the
     padding mask is applied as a per-partition bias on the Exp
     activation -- no mask matmuls.
  3. Denominator: V gets an appended ones column; the PV matmul's extra
     output column accumulates sum_k e_k = softmax denominator.
  4. All matmul operands bf16 (1 cycle/row on PE, same rate as fp32r,
     but halves SBUF/DMA so everything stays resident; rel err ~8e-3,
     well inside the 2e-2 gate).
  5. Overlap: weight/x DMAs interleaved per d-chunk so the first matmul
     starts ~2us in; one shared single-bank PSUM ring across all phases
     (no phase-boundary barriers); final scale+store split in halves.

Softmax shift-invariance: reference subtracts rowmax; we subtract
nothing (scores are O(10); exp in fp32 is safe) -- identical result.
bq/bk support: ones-row matmul folded into the psum accumulation;
bv added on host (softmax rows sum to 1).
"""

import sys

sys.path.insert(0, "/opt/trn_rl_repo")

import numpy as np
import ml_dtypes

import concourse.bacc as bacc
import concourse.tile as tile
from concourse import mybir
from concourse.bass_utils import run_bass_kernel_spmd

BF16 = mybir.dt.bfloat16
FP32 = mybir.dt.float32

S = 2048
D = 1024
NCORES = 8
SK_DEFAULT = 1152   # padded unmasked-key count (counts ~1024 +- 35)
NEC = D // 128      # 8 e-chunks
NDC = D // 128      # 8 d-chunks (contraction)
NQC = S // 128      # 16 query chunks
SCALE = 1.0 / np.sqrt(np.float32(D))
PAD_BIAS = -50.0    # exp(scale*0 + PAD_BIAS) == 2e-22: kills padding slots


def _nblocks(n, b=512):
    out = []
    o = 0
    while o < n:
        out.append((o, min(b, n - o)))
        o += b
    return out


def build_nc(has_bq: bool, has_bk: bool, repeat: int = 1, sk: int = SK_DEFAULT):
    assert sk % 128 == 0
    nkc = sk // 128

    nc = bacc.Bacc("TRN2", target_bir_lowering=False)

    XT = nc.dram_tensor("XT", [D, S], BF16, kind="ExternalInput")
    XGT = nc.dram_tensor("XGT", [D, sk], BF16, kind="ExternalInput")
    WQT = nc.dram_tensor("WQT", [D, D], BF16, kind="ExternalInput")
    WKT = nc.dram_tensor("WKT", [D, D], BF16, kind="ExternalInput")
    WVT = nc.dram_tensor("WVT", [D, D], BF16, kind="ExternalInput")
    MB = nc.dram_tensor("MB", [128, nkc], FP32, kind="ExternalInput")
    BQ = nc.dram_tensor("BQ", [1, D], BF16, kind="ExternalInput")
    BK = nc.dram_tensor("BK", [1, D], BF16, kind="ExternalInput")
    OUT = nc.dram_tensor("OUT", [S, D], FP32, kind="ExternalOutput")

    Copy = mybir.ActivationFunctionType.Copy
    Exp = mybir.ActivationFunctionType.Exp

    with tile.TileContext(nc) as tc:
        with (
            tc.tile_pool(name="const", bufs=1) as constp,
            tc.tile_pool(name="resp", bufs=1) as resp,
        ):
            mb = constp.tile([128, nkc], FP32)
            nc.sync.dma_start(mb, MB[:, :])
            if has_bq or has_bk:
                ones_row = constp.tile([1, S], BF16)
                nc.vector.memset(ones_row, 1.0)
                bq_sb = constp.tile([1, D], BF16)
                bk_sb = constp.tile([1, D], BF16)
                if has_bq:
                    nc.sync.dma_start(bq_sb, BQ[:, :])
                if has_bk:
                    nc.sync.dma_start(bk_sb, BK[:, :])

            # whole-kernel residents (bf16)
            qt = resp.tile([128, NEC, S], BF16)
            kt = resp.tile([128, NEC, sk], BF16)
            v = resp.tile([128, nkc, D + 1], BF16)

            for rep_i in range(repeat):
                with (
                    tc.tile_pool(name="xtp", bufs=1) as xtp,
                    tc.tile_pool(name="wp", bufs=2) as wp,
                    tc.tile_pool(name="ps", bufs=8, space="PSUM") as psp,
                ):
                    xt = xtp.tile([128, NDC, S], BF16, name=f"xt_{rep_i}")
                    xg = xtp.tile([128, NDC, sk], BF16, name=f"xg_{rep_i}")
                    wq_sb = wp.tile([128, NDC, D], BF16, tag="w", name=f"wq_{rep_i}")
                    wk_sb = wp.tile([128, NDC, D], BF16, tag="w", name=f"wk_{rep_i}")
                    # interleave so the first matmul's operands land first
                    for d in range(NDC):
                        nc.sync.dma_start(wq_sb[:, d, :], WQT[d * 128:(d + 1) * 128, :])
                        nc.sync.dma_start(xt[:, d, :], XT[d * 128:(d + 1) * 128, :])
                    for d in range(NDC):
                        nc.sync.dma_start(wk_sb[:, d, :], WKT[d * 128:(d + 1) * 128, :])
                        nc.sync.dma_start(xg[:, d, :], XGT[d * 128:(d + 1) * 128, :])

                    def proj_qk(w_sb, rhs, dst, has_b, b_sb, pname):
                        blocks = _nblocks(rhs.shape[-1])
                        for ec in range(NEC):
                            for (o, n) in blocks:
                                ps = psp.tile([128, 512], FP32, tag="ps",
                                              name=f"ps{pname}{ec}_{o}_{rep_i}")
                                for d in range(NDC):
                                    nc.tensor.matmul(
                                        ps[:, 0:n],
                                        w_sb[:, d, ec * 128:(ec + 1) * 128],
                                        rhs[:, d, o:o + n],
                                        start=(d == 0),
                                        stop=(d == NDC - 1 and not has_b),
                                    )
                                if has_b:
                                    nc.tensor.matmul(
                                        ps[:, 0:n],
                                        b_sb[0:1, ec * 128:(ec + 1) * 128],
                                        ones_row[0:1, o:o + n],
                                        start=False, stop=True,
                                    )
                                nc.scalar.activation(
                                    out=dst[:, ec, o:o + n], in_=ps[:, 0:n],
                                    func=Copy)

                    proj_qk(wq_sb, xt, qt, has_bq,
                            None if not has_bq else bq_sb, "q")
                    proj_qk(wk_sb, xg, kt, has_bk,
                            None if not has_bk else bk_sb, "k")

                    # V: gathered x^T chunk stationary, W_v moving
                    wv_sb = wp.tile([128, NDC, D], BF16, tag="w", name=f"wv_{rep_i}")
                    for d in range(NDC):
                        nc.sync.dma_start(wv_sb[:, d, :], WVT[d * 128:(d + 1) * 128, :])
                    for kc in range(nkc):
                        for eb in range(2):
                            ps = psp.tile([128, 512], FP32, tag="ps",
                                          name=f"psv{kc}_{eb}_{rep_i}")
                            for d in range(NDC):
                                nc.tensor.matmul(
                                    ps,
                                    xg[:, d, kc * 128:(kc + 1) * 128],
                                    wv_sb[:, d, eb * 512:(eb + 1) * 512],
                                    start=(d == 0),
                                    stop=(d == NDC - 1),
                                )
                            nc.scalar.activation(
                                out=v[:, kc, eb * 512:(eb + 1) * 512], in_=ps,
                                func=Copy)
                    nc.vector.memset(v[:, :, D:D + 1], 1.0)

                    # ---- Phase B: scores^T -> exp -> E^T resident ----
                    with tc.tile_pool(name="etp", bufs=1) as etp:
                        et = etp.tile([128, nkc, S], BF16, name=f"et_{rep_i}")
                        for kc in range(nkc):
                            for qb in range(S // 512):
                                ps = psp.tile([128, 512], FP32, tag="ps",
                                              name=f"pss{kc}_{qb}_{rep_i}")
                                for ec in range(NEC):
                                    nc.tensor.matmul(
                                        ps,
                                        kt[:, ec, kc * 128:(kc + 1) * 128],
                                        qt[:, ec, qb * 512:(qb + 1) * 512],
                                        start=(ec == 0),
                                        stop=(ec == NEC - 1),
                                    )
                                nc.scalar.activation(
                                    out=et[:, kc, qb * 512:(qb + 1) * 512],
                                    in_=ps, func=Exp,
                                    scale=float(SCALE), bias=mb[:, kc:kc + 1],
                                )

                        # ---- PV + denominator ----
                        with (
                            tc.tile_pool(name="outp", bufs=3) as outp,
                            tc.tile_pool(name="smallp", bufs=3) as smallp,
                        ):
                            for qc in range(NQC):
                                ps_o = [psp.tile([128, 512], FP32, tag="ps",
                                                 name=f"pso{qc}_{eb}_{rep_i}")
                                        for eb in range(2)]
                                ps_d = psp.tile([128, 512], FP32, tag="ps",
                                                name=f"psd{qc}_{rep_i}")
                                for kc in range(nkc):
                                    lhsT = et[:, kc, qc * 128:(qc + 1) * 128]
                                    nc.tensor.matmul(
                                        ps_d[:, 0:1], lhsT, v[:, kc, D:D + 1],
                                        start=(kc == 0), stop=(kc == nkc - 1),
                                    )
                                    for eb in range(2):
                                        nc.tensor.matmul(
                                            ps_o[eb], lhsT,
                                            v[:, kc, eb * 512:(eb + 1) * 512],
                                            start=(kc == 0), stop=(kc == nkc - 1),
                                        )
                                recip = smallp.tile([128, 1], FP32, tag="recip",
                                                    name=f"rc{qc}_{rep_i}")
                                nc.vector.reciprocal(recip, ps_d[:, 0:1])
                                for eb in range(2):
                                    osb = outp.tile([128, 512], FP32, tag="osb",
                                                    name=f"osb{qc}_{eb}_{rep_i}")
                                    nc.vector.tensor_scalar_mul(
                                        osb, ps_o[eb], recip)
                                    nc.sync.dma_start(
                                        OUT[qc * 128:(qc + 1) * 128,
                                            eb * 512:(eb + 1) * 512], osb)

    nc.compile()
    return nc


_NC_CACHE = {}


def _pick_sk(mask):
    """Smallest supported padded key count covering every batch's count."""
    counts = (np.asarray(mask) != 0).sum(axis=1)
    mx = int(counts.max())
    sk = max(SK_DEFAULT, ((mx + 127) // 128) * 128)
    return min(sk, S), counts


def _build_in_maps(inputs, sk=None):
    bf = ml_dtypes.bfloat16
    x = np.asarray(inputs["x"], dtype=np.float32)
    mask = np.asarray(inputs["mask"])
    Wq = np.asarray(inputs["Wq"], dtype=np.float32)
    Wk = np.asarray(inputs["Wk"], dtype=np.float32)
    Wv = np.asarray(inputs["Wv"], dtype=np.float32)
    bq = np.asarray(inputs.get("bq", np.zeros(D)), dtype=np.float32)
    bk = np.asarray(inputs.get("bk", np.zeros(D)), dtype=np.float32)
    if sk is None:
        sk, _ = _pick_sk(mask)
    nkc = sk // 128

    WQT = np.ascontiguousarray(Wq.T).astype(bf)
    WKT = np.ascontiguousarray(Wk.T).astype(bf)
    WVT = np.ascontiguousarray(Wv.T).astype(bf)
    bq_r = bq.reshape(1, D).astype(bf)
    bk_r = bk.reshape(1, D).astype(bf)

    in_maps = []
    for b in range(x.shape[0]):
        idx = np.nonzero(mask[b])[0]
        c = len(idx)
        assert c <= sk
        xg = np.zeros((sk, D), np.float32)
        xg[:c] = x[b][idx]
        mb = np.zeros(sk, np.float32)
        mb[c:] = PAD_BIAS
        in_maps.append({
            "XT": np.ascontiguousarray(x[b].T).astype(bf),
            "XGT": np.ascontiguousarray(xg.T).astype(bf),
            "WQT": WQT, "WKT": WKT, "WVT": WVT,
            "MB": np.ascontiguousarray(mb.reshape(nkc, 128).T),
            "BQ": bq_r, "BK": bk_r,
        })
    return in_maps


def _cpu_reference_batch(x_b, mask_b, Wq, bq, Wk, bk, Wv, bv):
    """Exact fp32 fallback for degenerate batches (e.g. all keys masked)."""
    q = x_b @ Wq.T + bq
    k = x_b @ Wk.T + bk
    vv = x_b @ Wv.T + bv
    s = (q @ k.T) / np.sqrt(np.float32(D))
    s = np.where(mask_b[None, :] == 0, np.float32(-1e9), s)
    s = s - s.max(axis=1, keepdims=True)
    e = np.exp(s)
    return (e @ vv) / e.sum(axis=1, keepdims=True)


def kernel(x, mask, Wq, bq, Wk, bk, Wv, bv):
    x = np.asarray(x, dtype=np.float32)
    mask = np.asarray(mask)
    bq = np.asarray(bq, dtype=np.float32)
    bk = np.asarray(bk, dtype=np.float32)
    bv = np.asarray(bv, dtype=np.float32)

    B = x.shape[0]
    assert x.shape == (B, S, D) and B == NCORES

    sk, counts = _pick_sk(mask)

    has_bq = bool(np.any(bq != 0.0))
    has_bk = bool(np.any(bk != 0.0))
    key = (has_bq, has_bk, sk)
    if key not in _NC_CACHE:
        _NC_CACHE[key] = build_nc(has_bq, has_bk, sk=sk)
    nc = _NC_CACHE[key]

    in_maps = _build_in_maps(
        {"x": x, "mask": mask, "Wq": Wq, "Wk": Wk, "Wv": Wv,
         "bq": bq, "bk": bk}, sk=sk)

    res = run_bass_kernel_spmd(nc, in_maps, core_ids=list(range(NCORES)))
    out = np.stack([res.results[b]["OUT"] for b in range(B)], axis=0)
    if np.any(bv != 0.0):
        out = out + bv[None, None, :]
    for b in range(B):
        if counts[b] == 0:
            out[b] = _cpu_reference_batch(
                x[b], mask[b], Wq, bq, Wk, bk, Wv, bv)
    return out.astype(np.float32)
